# revision 13
# baseline (speedup 1.0000x reference)
"""Single-head attention (B=16, S=1024, D=768) on 8 Trainium2 NeuronCores.

Sharding: data-parallel over batch — each core computes 2 full batches with
all weights replicated. No collectives.

Layout strategy (all matmul operands float32r — full PE rate; measured
faster per-row than bf16 on this silicon):
  - x is host-transposed/pre-tiled to [BL, th, kt, 128, 512] so each
    [128, 512] d-block is one contiguous-DRAM DMA; the 12 x DMAs per batch
    alternate between the sync HWDGE queue (even kt) and the gpsimd SWDGE
    queue (odd kt) so two queues generate/transfer in parallel — startup is
    input-DMA bound.
  - weights are DMA'd ONCE (resident in SBUF across both batches): wqk
    eh-slices + wf on the scalar HWDGE queue in first-use order (the first
    slice is further kt-split so the very first chain starts sooner); the
    last two wqk slices ride the gpsimd queue after the x slices it carries.
  - q, k are folded: S = x @ (s*w_q^T @ w_k) @ x^T (A-Z stage produces
    Z^T = (x @ wqkf)^T directly).
  - the output projection is folded into the value projection
    (wf = w_out @ w_v): vw = x @ wf^T token-major, y^T = P @ vw — one
    matmul stage instead of two, 15% fewer FLOPs.
  - S is computed TRANSPOSED ([j, i]) so exp(S) lands in the layout the
    P-contraction needs; scale 1/sqrt(D) and biases are folded on the host
    (colterm = x @ u emerges as column 768 of the vw matmul, exactly the
    per-partition exp bias layout).
  - softmax denominator via a DVE pairwise add tree + gpsimd cross-partition
    all-reduce off the PE critical path; the very last tile uses PE row-sum
    matmuls instead (no PE work left to hide behind).
  - PE warmup matmuls on a memset tile (no DMA dependency) keep the PE busy
    from the end of the framework preamble until the first x slice lands,
    so the HAM clock gate ramps before real work starts.
"""

import sys

import numpy as np

if "/opt/trn_rl_repo" not in sys.path:
    sys.path.insert(0, "/opt/trn_rl_repo")

import ml_dtypes  # noqa: E402

import concourse.bass_isa as bass_isa  # noqa: E402
import concourse.mybir as mybir  # noqa: E402
import concourse.tile as tile  # noqa: E402
from concourse import bacc  # noqa: E402
from concourse.bass_interp import get_hw_module  # noqa: E402
from concourse.bass_utils import run_bass_kernel_spmd  # noqa: E402

N_CORES = 8
B, S, D = 16, 1024, 768
BL = B // N_CORES  # batches per core
KT = D // 128  # 6 contraction tiles
F32 = mybir.dt.float32
F32R = mybir.dt.float32r
BF16 = mybir.dt.bfloat16
WARMUP = 20

_prog = None


def _build():
    nc = bacc.Bacc("TRN2", target_bir_lowering=False, debug=False, num_devices=N_CORES)
    # pre-tiled on host: every DMA reads one contiguous DRAM block into
    # 128 per-partition rows (minimal descriptor count, full DMA bandwidth)
    xT_d = nc.dram_tensor("xTt", [BL, 2, KT, 128, 512], F32R,
                          kind="ExternalInput").ap()
    wqk_d = nc.dram_tensor("wqkt", [3, 2, 128, KT, 128], F32R,
                           kind="ExternalInput").ap()
    onc_d = nc.dram_tensor("onesc", [128, 1], F32R, kind="ExternalInput").ap()
    wf_d = nc.dram_tensor("wft", [2, 128, KT, 386], F32R,
                          kind="ExternalInput").ap()
    boute_d = nc.dram_tensor("boute", [128, KT], F32, kind="ExternalInput").ap()
    y_d = nc.dram_tensor("y", [BL, D, S], F32, kind="ExternalOutput").ap()

    Exp = mybir.ActivationFunctionType.Exp
    Mult = mybir.AluOpType.mult
    Add = mybir.AluOpType.add

    with tile.TileContext(nc) as tc:
        with tc.tile_pool(name="consts", bufs=1) as consts, \
             tc.tile_pool(name="wqk", bufs=6) as wqkp, \
             tc.tile_pool(name="wf", bufs=2) as wfp, \
             tc.tile_pool(name="xT", bufs=4) as xp, \
             tc.tile_pool(name="qk", bufs=1) as qkp, \
             tc.tile_pool(name="vw", bufs=1) as vwp, \
             tc.tile_pool(name="pt", bufs=2) as ptp, \
             tc.tile_pool(name="y", bufs=3) as yp, \
             tc.tile_pool(name="small", bufs=1) as smallp, \
             tc.tile_pool(name="mm", bufs=8, space="PSUM") as mmp:

            boute_sb = consts.tile([128, KT], F32)
            onc_sb = consts.tile([128, 1], F32R)

            # PE warmup on a memset tile: no DMA dependency, so the PE is
            # busy from the end of the framework preamble until the first
            # real inputs land — the HAM clock gate ramps before real work
            warm_sb = consts.tile([128, 128], BF16)
            nc.vector.memset(warm_sb[:], 0.25)
            wps = mmp.tile([128, 512], F32, tag="mm", name="warmps")
            for i in range(WARMUP):
                nc.tensor.matmul(wps[:, :128], warm_sb[:], warm_sb[:],
                                 start=True, stop=True)

            # weights: loaded once, resident for both batches.
            # scalar queue: wsl00 (kt-split for the earliest possible first
            # chain), wsl01, wsl10, wsl11, then wf0, wf1, consts.
            # gpsimd queue carries wsl20/wsl21 (needed later) between the
            # b=0 x-odd slices and the b=1 ones.
            wsl = [[wqkp.tile([128, KT, 128], F32R, tag="wqk",
                              name=f"wsl{ew}_{eh}") for eh in range(2)]
                   for ew in range(3)]
            # scalar queue: wqk slices in first-use order (first slice
            # kt-split so the very first chain starts sooner), then consts
            nc.scalar.dma_start(wsl[0][0][:, :3], wqk_d[0, 0, :, :3])
            nc.scalar.dma_start(wsl[0][0][:, 3:], wqk_d[0, 0, :, 3:])
            nc.scalar.dma_start(wsl[0][1][:], wqk_d[0, 1])
            nc.scalar.dma_start(wsl[1][0][:], wqk_d[1, 0])
            nc.scalar.dma_start(wsl[1][1][:], wqk_d[1, 1])
            nc.scalar.dma_start(wsl[2][0][:], wqk_d[2, 0])
            nc.gpsimd.dma_start(wsl[2][1][:], wqk_d[2, 1])
            nc.scalar.dma_start(onc_sb[:], onc_d[:])
            nc.scalar.dma_start(boute_sb[:], boute_d[:])
            wfs = [wfp.tile([128, KT, 386], F32R, tag="wf", name=f"wf{f2}")
                   for f2 in range(2)]

            xhs = {}
            for b in range(BL):
                xhs[b] = [xp.tile([128, KT, 512], F32R, tag="xT",
                                  name=f"xh{b}_{h}") for h in range(2)]

            # b=0 x: all 12 slices on the sync queue in consumption order,
            # with the wf halves spliced in between the two th halves
            # (wf is needed by the A-vw half that runs between them)
            for kt in range(KT):
                nc.sync.dma_start(xhs[0][0][:, kt], xT_d[0, 0, kt])
            nc.sync.dma_start(wfs[0][:, :3], wf_d[0, :, :3])
            nc.sync.dma_start(wfs[0][:, 3:], wf_d[0, :, 3:])
            nc.sync.dma_start(wfs[1][:, :3], wf_d[1, :, :3])
            nc.sync.dma_start(wfs[1][:, 3:], wf_d[1, :, 3:])
            for kt in range(KT):
                nc.sync.dma_start(xhs[0][1][:, kt], xT_d[0, 1, kt])
            # b=1 x: prefetched behind the b=0 traffic — th0 rides the
            # scalar queue (idle after the weights), th1 the sync queue
            for kt in range(KT):
                nc.scalar.dma_start(xhs[1][0][:, kt], xT_d[1, 0, kt])
            for kt in range(KT):
                nc.sync.dma_start(xhs[1][1][:, kt], xT_d[1, 1, kt])

            for b in range(BL):
                xh = xhs[b]
                ZT = qkp.tile([128, KT, S], F32R, tag="ZT")
                vw_sb = vwp.tile([128, 8, D + 2], F32R, tag="vw")

                def fill(n):
                    # clock-keeper matmuls: the startup is input-DMA bound,
                    # and PE idle gaps there drop the HAM clock to 1.2 GHz
                    # for the next ~10us; these no-dependency fillers keep
                    # the PE duty cycle high while slices land
                    for _ in range(n):
                        nc.tensor.matmul(wps[:, :128], warm_sb[:], warm_sb[:],
                                         start=True, stop=True)

                def az_chains(th, fkt=0, fch=0):
                    # A-Z: Z^T[e, t] for Z = x @ (s*w_q^T @ w_k); S = Z @ x^T
                    for ew in range(3):
                        for eh in range(2):
                            et = 2 * ew + eh
                            ps = mmp.tile([128, 512], F32, tag="mm")
                            for kt in range(KT):
                                nc.tensor.matmul(ps[:], wsl[ew][eh][:, kt],
                                                 xh[th][:, kt],
                                                 start=(kt == 0), stop=(kt == KT - 1))
                                if et == 0 and kt < KT - 1:
                                    fill(fkt)
                            nc.scalar.copy(ZT[:, et, 512 * th:512 * (th + 1)], ps[:])
                            fill(fch)

                def avw_chains(tth, fch=(0, 0)):
                    # A-vw: vw[t, e] = x @ [wf | u]^T token-major. Column 768
                    # is colterm[t] = x @ u — the surviving softmax bias, in
                    # exactly the per-partition layout the exp ACT bias needs
                    for f2, (foff, fsz) in enumerate(((0, 384), (384, 386))):
                        for tt in range(4 * tth, 4 * tth + 4):
                            ps = mmp.tile([128, 512], F32, tag="mm")
                            for kt in range(KT):
                                nc.tensor.matmul(ps[:, :fsz],
                                                 xh[tt // 4][:, kt, 128 * (tt % 4):128 * (tt % 4 + 1)],
                                                 wfs[f2][:, kt, :fsz],
                                                 start=(kt == 0), stop=(kt == KT - 1))
                            nc.vector.tensor_copy(vw_sb[:, tt, foff:foff + fsz],
                                                  ps[:, :fsz])
                            fill(fch[f2])

                # th0-only work first so the th1 x slices have time to land;
                # fillers only where the measured trace shows DMA starvation
                if b == 0:
                    az_chains(0, fkt=3, fch=5)
                    avw_chains(0, fch=(4, 1))
                    az_chains(1)
                    avw_chains(1)
                else:
                    az_chains(0)
                    avw_chains(0)
                    az_chains(1)
                    avw_chains(1)

                for ih in range(2):
                    # B: S^T[j, i] tiles -> exp -> PT (unnormalized)
                    PT = ptp.tile([128, 8, 512], F32R, tag="PT")
                    for jt in range(8):
                        ps = mmp.tile([128, 512], F32, tag="mm")
                        for dt in range(KT):
                            nc.tensor.matmul(ps[:], xh[jt // 4][:, dt, 128 * (jt % 4):128 * (jt % 4 + 1)],
                                             ZT[:, dt, 512 * ih:512 * (ih + 1)],
                                             start=(dt == 0), stop=(dt == KT - 1))
                        nc.scalar.activation(PT[:, jt], ps[:], Exp,
                                             bias=vw_sb[:, jt, D:D + 1])

                    # C: softmax denominator. Mid-kernel the DVE add tree +
                    # gpsimd all-reduce hides behind PE work; for the very
                    # last tile there is no PE work left to hide behind, so
                    # use PE row-sum matmuls (ready right after the last exp)
                    rb = smallp.tile([128, 512], F32, tag="rb")
                    if b == BL - 1 and ih == 1:
                        pr = mmp.tile([128, 512], F32, tag="mm", name="sumrow")
                        for jt in range(8):
                            nc.tensor.matmul(pr[0:1, :], onc_sb[:], PT[:, jt],
                                             start=(jt == 0), stop=(jt == 7))
                        rrow = smallp.tile([1, 512], F32, tag="rrow")
                        nc.vector.reciprocal_approx_fast(rrow[0:1, :], pr[0:1, :])
                        nc.gpsimd.partition_broadcast(rb[:], rrow[0:1, :])
                    else:
                        tree = smallp.tile([128, 4, 512], F32, tag="tree")
                        for p in range(4):
                            nc.vector.tensor_tensor(tree[:, p], PT[:, 2 * p],
                                                    PT[:, 2 * p + 1], Add)
                        nc.vector.tensor_tensor(tree[:, 0], tree[:, 0], tree[:, 1], Add)
                        nc.vector.tensor_tensor(tree[:, 2], tree[:, 2], tree[:, 3], Add)
                        nc.vector.tensor_tensor(tree[:, 1], tree[:, 0], tree[:, 2], Add)
                        nc.gpsimd.partition_all_reduce(tree[:, 3], tree[:, 1], 128,
                                                       bass_isa.ReduceOp.add)
                        nc.vector.reciprocal_approx_fast(rb[:], tree[:, 3])

                    # D: y^T[e, i] = (vw^T @ P^T) * (1/denom) + b_out_eff
                    for et in range(KT):
                        ps = mmp.tile([128, 512], F32, tag="mm")
                        for jt in range(8):
                            nc.tensor.matmul(ps[:], vw_sb[:, jt, 128 * et:128 * (et + 1)],
                                             PT[:, jt], start=(jt == 0), stop=(jt == 7))
                        yt = yp.tile([128, 512], F32, tag="y")
                        if b == BL - 1 and ih == 1 and et == KT - 1:
                            # very last tile: halve the post-chain so the
                            # final DMA starts sooner (shorter drain tail)
                            for ho in (0, 256):
                                sl = slice(ho, ho + 256)
                                nc.vector.tensor_tensor(yt[:, sl], ps[:, sl],
                                                        rb[:, sl], Mult)
                                nc.vector.tensor_scalar_add(yt[:, sl], yt[:, sl],
                                                            boute_sb[:, et:et + 1])
                                nc.scalar.dma_start(
                                    y_d[b, 128 * et:128 * (et + 1),
                                        512 * ih + ho:512 * ih + ho + 256],
                                    yt[:, sl])
                        else:
                            nc.vector.tensor_tensor(yt[:], ps[:], rb[:], Mult)
                            nc.vector.tensor_scalar_add(yt[:], yt[:],
                                                        boute_sb[:, et:et + 1])
                            nc.scalar.dma_start(
                                y_d[b, 128 * et:128 * (et + 1),
                                    512 * ih:512 * (ih + 1)],
                                yt[:])

    nc.compile()
    nc.m = get_hw_module(nc.m)
    return nc


def _prepare_in_maps(x, w_qkv, b_qkv, w_out, b_out):
    x = np.asarray(x, dtype=np.float32)
    w_qkv = np.asarray(w_qkv, dtype=np.float32)
    b_qkv = np.asarray(b_qkv, dtype=np.float32)
    w_out = np.asarray(w_out, dtype=np.float32)
    b_out = np.asarray(b_out, dtype=np.float32)

    s = D ** -0.5
    w_q = w_qkv[:D, :]
    w_k = w_qkv[D:2 * D, :]
    w_v = w_qkv[2 * D:, :]
    # folded score projection: S = x @ wqkf @ x^T with wqkf = s*w_q^T @ w_k
    wqkf = (s * w_q.T) @ w_k  # [d_in, d_out]
    # only surviving score bias: colterm = x @ u, u = w_k^T @ (s*b_q)
    u = w_k.T @ (s * b_qkv[:D])  # [D]
    # folded value/output projection, augmented with u as a 769th column so
    # colterm falls out of the vw matmul for free; 770th column zero-pads
    wf = w_out @ w_v  # [D, D]
    wf_aug = np.concatenate(
        [wf.T, u[:, None], np.zeros((D, 1), np.float32)], axis=1)  # [d, D+2]
    b_out_eff = (b_out + w_out @ b_qkv[2 * D:]).astype(np.float32)
    boute_arr = np.ascontiguousarray(b_out_eff.reshape(KT, 128).T)  # [128, KT]
    # stationary weights: wsl[ew][eh][p, kt, j] = wqkf[kt*128+p, (2ew+eh)*128+j]
    wqk_t = np.ascontiguousarray(
        wqkf.reshape(KT, 128, 6, 128).transpose(2, 1, 0, 3).reshape(
            3, 2, 128, KT, 128).astype(np.float32))
    wf_t = np.zeros((2, 128, KT, 386), np.float32)
    for f2, (foff, fsz) in enumerate(((0, 384), (384, 386))):
        wf_t[f2, :, :, :fsz] = wf_aug[:, foff:foff + fsz].reshape(
            KT, 128, fsz).transpose(1, 0, 2)

    in_maps = []
    for c in range(N_CORES):
        xl = x[BL * c:BL * (c + 1)]
        xT = xl.transpose(0, 2, 1)  # [BL, D, S]
        # [BL, th, kt, 128, 512]: one contiguous [128, 512] block per DMA
        xT_t = np.ascontiguousarray(
            xT.reshape(BL, KT, 128, 2, 512).transpose(0, 3, 1, 2, 4))
        in_maps.append({
            "xTt": xT_t, "wqkt": wqk_t, "wft": wf_t,
            "onesc": np.ones((128, 1), np.float32),
            "boute": boute_arr,
        })
    return in_maps


def _get_prog():
    global _prog
    if _prog is None:
        _prog = _build()
    return _prog


def _run(in_maps, **kwargs):
    res = run_bass_kernel_spmd(_get_prog(), in_maps, list(range(N_CORES)), **kwargs)
    return res


def kernel(x, w_qkv, b_qkv, w_out, b_out):
    in_maps = _prepare_in_maps(x, w_qkv, b_qkv, w_out, b_out)
    res = _run(in_maps)
    # kernel produces y transposed ([BL, D, S]); transpose back on host
    y = np.concatenate(
        [res.results[c]["y"].transpose(0, 2, 1) for c in range(N_CORES)], axis=0)
    return np.ascontiguousarray(y).astype(np.float32)


# revision 15
# speedup vs baseline: 1.3004x; 1.3004x over previous
"""Single-head attention (B=16, S=1024, D=768) on 8 Trainium2 NeuronCores.

Sharding: data-parallel over batch — each core computes 2 full batches with
all weights replicated. No collectives.

Layout strategy (all matmul operands float32r — full PE rate; measured
faster per-row than bf16 on this silicon):
  - x is host-transposed/pre-tiled to [BL, th, kt, 128, 512] so each
    [128, 512] d-block is one contiguous-DRAM DMA; the 12 x DMAs per batch
    alternate between the sync HWDGE queue (even kt) and the gpsimd SWDGE
    queue (odd kt) so two queues generate/transfer in parallel — startup is
    input-DMA bound.
  - weights are DMA'd ONCE (resident in SBUF across both batches): wqk
    eh-slices + wf on the scalar HWDGE queue in first-use order (the first
    slice is further kt-split so the very first chain starts sooner); the
    last two wqk slices ride the gpsimd queue after the x slices it carries.
  - q, k are folded: S = x @ (s*w_q^T @ w_k) @ x^T (A-Z stage produces
    Z^T = (x @ wqkf)^T directly).
  - the output projection is folded into the value projection
    (wf = w_out @ w_v): vw = x @ wf^T token-major, y^T = P @ vw — one
    matmul stage instead of two, 15% fewer FLOPs.
  - S is computed TRANSPOSED ([j, i]) so exp(S) lands in the layout the
    P-contraction needs; scale 1/sqrt(D) and biases are folded on the host
    (colterm = x @ u emerges as column 768 of the vw matmul, exactly the
    per-partition exp bias layout).
  - softmax denominator via a DVE pairwise add tree + gpsimd cross-partition
    all-reduce off the PE critical path; the very last tile uses PE row-sum
    matmuls instead (no PE work left to hide behind).
  - PE warmup matmuls on a memset tile (no DMA dependency) keep the PE busy
    from the end of the framework preamble until the first x slice lands,
    so the HAM clock gate ramps before real work starts.
"""

import sys

import numpy as np

if "/opt/trn_rl_repo" not in sys.path:
    sys.path.insert(0, "/opt/trn_rl_repo")

import ml_dtypes  # noqa: E402

import concourse.bass_isa as bass_isa  # noqa: E402
import concourse.mybir as mybir  # noqa: E402
import concourse.tile as tile  # noqa: E402
from concourse import bacc  # noqa: E402
from concourse.bass_interp import get_hw_module  # noqa: E402
from concourse.bass_utils import run_bass_kernel_spmd  # noqa: E402

N_CORES = 8
B, S, D = 16, 1024, 768
BL = B // N_CORES  # batches per core
KT = D // 128  # 6 contraction tiles
F32 = mybir.dt.float32
F32R = mybir.dt.float32r
BF16 = mybir.dt.bfloat16
WARMUP = 18

_prog = None


def _build():
    nc = bacc.Bacc("TRN2", target_bir_lowering=False, debug=False, num_devices=N_CORES)
    # pre-tiled on host: every DMA reads one contiguous DRAM block into
    # 128 per-partition rows (minimal descriptor count, full DMA bandwidth)
    xT_d = nc.dram_tensor("xTt", [BL, 2, KT, 128, 512], BF16,
                          kind="ExternalInput").ap()
    wqk_d = nc.dram_tensor("wqkt", [3, 2, 128, KT, 128], BF16,
                           kind="ExternalInput").ap()
    onc_d = nc.dram_tensor("onesc", [128, 1], BF16, kind="ExternalInput").ap()
    wf_d = nc.dram_tensor("wft", [2, 128, KT, 386], BF16,
                          kind="ExternalInput").ap()
    boute_d = nc.dram_tensor("boute", [128, KT], F32, kind="ExternalInput").ap()
    y_d = nc.dram_tensor("y", [BL, D, S], BF16, kind="ExternalOutput").ap()

    Exp = mybir.ActivationFunctionType.Exp
    Mult = mybir.AluOpType.mult
    Add = mybir.AluOpType.add

    with tile.TileContext(nc) as tc:
        with tc.tile_pool(name="consts", bufs=1) as consts, \
             tc.tile_pool(name="wqk", bufs=6) as wqkp, \
             tc.tile_pool(name="wf", bufs=2) as wfp, \
             tc.tile_pool(name="xT", bufs=4) as xp, \
             tc.tile_pool(name="qk", bufs=1) as qkp, \
             tc.tile_pool(name="vw", bufs=1) as vwp, \
             tc.tile_pool(name="pt", bufs=2) as ptp, \
             tc.tile_pool(name="y", bufs=3) as yp, \
             tc.tile_pool(name="small", bufs=1) as smallp, \
             tc.tile_pool(name="mm", bufs=8, space="PSUM") as mmp:

            boute_sb = consts.tile([128, KT], F32)
            onc_sb = consts.tile([128, 1], BF16)

            # PE warmup on a memset tile: no DMA dependency, so the PE is
            # busy from the end of the framework preamble until the first
            # real inputs land — the HAM clock gate ramps before real work
            warm_sb = consts.tile([128, 128], BF16)
            nc.vector.memset(warm_sb[:], 0.25)
            wps = mmp.tile([128, 512], F32, tag="mm", name="warmps")
            for i in range(WARMUP):
                nc.tensor.matmul(wps[:, :128], warm_sb[:], warm_sb[:],
                                 start=True, stop=True)

            # weights: loaded once, resident for both batches.
            # scalar queue: wsl00 (kt-split for the earliest possible first
            # chain), wsl01, wsl10, wsl11, then wf0, wf1, consts.
            # gpsimd queue carries wsl20/wsl21 (needed later) between the
            # b=0 x-odd slices and the b=1 ones.
            wsl = [[wqkp.tile([128, KT, 128], BF16, tag="wqk",
                              name=f"wsl{ew}_{eh}") for eh in range(2)]
                   for ew in range(3)]
            # scalar queue: wqk slices in first-use order (first slice
            # kt-split so the very first chain starts sooner), then consts
            nc.scalar.dma_start(wsl[0][0][:, :3], wqk_d[0, 0, :, :3])
            nc.scalar.dma_start(wsl[0][0][:, 3:], wqk_d[0, 0, :, 3:])
            nc.scalar.dma_start(wsl[0][1][:], wqk_d[0, 1])
            nc.scalar.dma_start(wsl[1][0][:], wqk_d[1, 0])
            nc.scalar.dma_start(wsl[1][1][:], wqk_d[1, 1])
            nc.scalar.dma_start(wsl[2][0][:], wqk_d[2, 0])
            nc.gpsimd.dma_start(wsl[2][1][:], wqk_d[2, 1])
            nc.scalar.dma_start(onc_sb[:], onc_d[:])
            nc.scalar.dma_start(boute_sb[:], boute_d[:])
            wfs = [wfp.tile([128, KT, 386], BF16, tag="wf", name=f"wf{f2}")
                   for f2 in range(2)]

            xhs = {}
            for b in range(BL):
                xhs[b] = [xp.tile([128, KT, 512], BF16, tag="xT",
                                  name=f"xh{b}_{h}") for h in range(2)]

            # b=0 x: all 12 slices on the sync queue in consumption order,
            # with the wf halves spliced in between the two th halves
            # (wf is needed by the A-vw half that runs between them)
            for kt in range(KT):
                nc.sync.dma_start(xhs[0][0][:, kt], xT_d[0, 0, kt])
            nc.sync.dma_start(wfs[0][:, :3], wf_d[0, :, :3])
            nc.sync.dma_start(wfs[0][:, 3:], wf_d[0, :, 3:])
            nc.sync.dma_start(wfs[1][:, :3], wf_d[1, :, :3])
            nc.sync.dma_start(wfs[1][:, 3:], wf_d[1, :, 3:])
            for kt in range(KT):
                nc.sync.dma_start(xhs[0][1][:, kt], xT_d[0, 1, kt])
            # b=1 x: prefetched behind the b=0 traffic — th0 rides the
            # scalar queue (idle after the weights), th1 the sync queue
            for kt in range(KT):
                nc.scalar.dma_start(xhs[1][0][:, kt], xT_d[1, 0, kt])
            for kt in range(KT):
                nc.sync.dma_start(xhs[1][1][:, kt], xT_d[1, 1, kt])

            for b in range(BL):
                xh = xhs[b]
                ZT = qkp.tile([128, KT, S], BF16, tag="ZT")
                vw_sb = vwp.tile([128, 8, D], BF16, tag="vw")
                colt = smallp.tile([128, 8, 1], F32, tag="colt")

                def fill(n):
                    # clock-keeper matmuls: the startup is input-DMA bound,
                    # and PE idle gaps there drop the HAM clock to 1.2 GHz
                    # for the next ~10us; these no-dependency fillers keep
                    # the PE duty cycle high while slices land
                    for _ in range(n):
                        nc.tensor.matmul(wps[:, :128], warm_sb[:], warm_sb[:],
                                         start=True, stop=True)

                def az_chains(th, fkt=0, fch=0):
                    # A-Z: Z^T[e, t] for Z = x @ (s*w_q^T @ w_k); S = Z @ x^T
                    for ew in range(3):
                        for eh in range(2):
                            et = 2 * ew + eh
                            ps = mmp.tile([128, 512], F32, tag="mm")
                            for kt in range(KT):
                                nc.tensor.matmul(ps[:], wsl[ew][eh][:, kt],
                                                 xh[th][:, kt],
                                                 start=(kt == 0), stop=(kt == KT - 1))
                                if et == 0 and kt < KT - 1:
                                    fill(fkt)
                            nc.scalar.copy(ZT[:, et, 512 * th:512 * (th + 1)], ps[:])
                            fill(fch)

                def avw_chains(tth, fch=(0, 0)):
                    # A-vw: vw[t, e] = x @ [wf | u]^T token-major. Column 768
                    # is colterm[t] = x @ u — the surviving softmax bias, in
                    # exactly the per-partition layout the exp ACT bias needs
                    for f2, (foff, fsz) in enumerate(((0, 384), (384, 386))):
                        for tt in range(4 * tth, 4 * tth + 4):
                            ps = mmp.tile([128, 512], F32, tag="mm")
                            for kt in range(KT):
                                nc.tensor.matmul(ps[:, :fsz],
                                                 xh[tt // 4][:, kt, 128 * (tt % 4):128 * (tt % 4 + 1)],
                                                 wfs[f2][:, kt, :fsz],
                                                 start=(kt == 0), stop=(kt == KT - 1))
                            if f2 == 0:
                                nc.vector.tensor_copy(vw_sb[:, tt, 0:384],
                                                      ps[:, :384])
                            else:
                                nc.vector.tensor_copy(vw_sb[:, tt, 384:768],
                                                      ps[:, :384])
                                nc.vector.tensor_copy(colt[:, tt], ps[:, 384:385])
                            fill(fch[f2])

                # th0-only work first so the th1 x slices have time to land
                az_chains(0)
                avw_chains(0)
                az_chains(1)
                avw_chains(1)

                for ih in range(2):
                    # B: S^T[j, i] tiles -> exp -> PT (unnormalized)
                    PT = ptp.tile([128, 8, 512], BF16, tag="PT")
                    for jt in range(8):
                        ps = mmp.tile([128, 512], F32, tag="mm")
                        for dt in range(KT):
                            nc.tensor.matmul(ps[:], xh[jt // 4][:, dt, 128 * (jt % 4):128 * (jt % 4 + 1)],
                                             ZT[:, dt, 512 * ih:512 * (ih + 1)],
                                             start=(dt == 0), stop=(dt == KT - 1))
                        nc.scalar.activation(PT[:, jt], ps[:], Exp,
                                             bias=colt[:, jt])

                    # C: softmax denominator. Mid-kernel the DVE add tree +
                    # gpsimd all-reduce hides behind PE work; for the very
                    # last tile there is no PE work left to hide behind, so
                    # use PE row-sum matmuls (ready right after the last exp)
                    rb = smallp.tile([128, 512], F32, tag="rb")
                    if b == BL - 1 and ih == 1:
                        pr = mmp.tile([128, 512], F32, tag="mm", name="sumrow")
                        for jt in range(8):
                            nc.tensor.matmul(pr[0:1, :], onc_sb[:], PT[:, jt],
                                             start=(jt == 0), stop=(jt == 7))
                        rrow = smallp.tile([1, 512], F32, tag="rrow")
                        nc.vector.reciprocal_approx_fast(rrow[0:1, :], pr[0:1, :])
                        nc.gpsimd.partition_broadcast(rb[:], rrow[0:1, :])
                    else:
                        tree = smallp.tile([128, 4, 512], F32, tag="tree")
                        for p in range(4):
                            nc.vector.tensor_tensor(tree[:, p], PT[:, 2 * p],
                                                    PT[:, 2 * p + 1], Add)
                        nc.vector.tensor_tensor(tree[:, 0], tree[:, 0], tree[:, 1], Add)
                        nc.vector.tensor_tensor(tree[:, 2], tree[:, 2], tree[:, 3], Add)
                        nc.vector.tensor_tensor(tree[:, 1], tree[:, 0], tree[:, 2], Add)
                        nc.gpsimd.partition_all_reduce(tree[:, 3], tree[:, 1], 128,
                                                       bass_isa.ReduceOp.add)
                        nc.vector.reciprocal_approx_fast(rb[:], tree[:, 3])

                    # D: y^T[e, i] = (vw^T @ P^T) * (1/denom) + b_out_eff
                    for et in range(KT):
                        ps = mmp.tile([128, 512], F32, tag="mm")
                        for jt in range(8):
                            nc.tensor.matmul(ps[:], vw_sb[:, jt, 128 * et:128 * (et + 1)],
                                             PT[:, jt], start=(jt == 0), stop=(jt == 7))
                        yt = yp.tile([128, 512], BF16, tag="y")
                        if b == BL - 1 and ih == 1 and et == KT - 1:
                            # very last tile: halve the post-chain so the
                            # final DMA starts sooner (shorter drain tail)
                            for ho in (0, 256):
                                sl = slice(ho, ho + 256)
                                nc.vector.tensor_tensor(yt[:, sl], ps[:, sl],
                                                        rb[:, sl], Mult)
                                nc.vector.tensor_scalar_add(yt[:, sl], yt[:, sl],
                                                            boute_sb[:, et:et + 1])
                                nc.scalar.dma_start(
                                    y_d[b, 128 * et:128 * (et + 1),
                                        512 * ih + ho:512 * ih + ho + 256],
                                    yt[:, sl])
                        else:
                            nc.vector.tensor_tensor(yt[:], ps[:], rb[:], Mult)
                            nc.vector.tensor_scalar_add(yt[:], yt[:],
                                                        boute_sb[:, et:et + 1])
                            nc.scalar.dma_start(
                                y_d[b, 128 * et:128 * (et + 1),
                                    512 * ih:512 * (ih + 1)],
                                yt[:])

    nc.compile()
    nc.m = get_hw_module(nc.m)
    return nc


def _prepare_in_maps(x, w_qkv, b_qkv, w_out, b_out):
    x = np.asarray(x, dtype=np.float32)
    w_qkv = np.asarray(w_qkv, dtype=np.float32)
    b_qkv = np.asarray(b_qkv, dtype=np.float32)
    w_out = np.asarray(w_out, dtype=np.float32)
    b_out = np.asarray(b_out, dtype=np.float32)

    s = D ** -0.5
    w_q = w_qkv[:D, :]
    w_k = w_qkv[D:2 * D, :]
    w_v = w_qkv[2 * D:, :]
    # folded score projection: S = x @ wqkf @ x^T with wqkf = s*w_q^T @ w_k
    wqkf = (s * w_q.T) @ w_k  # [d_in, d_out]
    # only surviving score bias: colterm = x @ u, u = w_k^T @ (s*b_q)
    u = w_k.T @ (s * b_qkv[:D])  # [D]
    # folded value/output projection, augmented with u as a 769th column so
    # colterm falls out of the vw matmul for free; 770th column zero-pads
    wf = w_out @ w_v  # [D, D]
    wf_aug = np.concatenate(
        [wf.T, u[:, None], np.zeros((D, 1), np.float32)], axis=1)  # [d, D+2]
    b_out_eff = (b_out + w_out @ b_qkv[2 * D:]).astype(np.float32)
    boute_arr = np.ascontiguousarray(b_out_eff.reshape(KT, 128).T)  # [128, KT]
    # stationary weights: wsl[ew][eh][p, kt, j] = wqkf[kt*128+p, (2ew+eh)*128+j]
    wqk_t = np.ascontiguousarray(
        wqkf.reshape(KT, 128, 6, 128).transpose(2, 1, 0, 3).reshape(
            3, 2, 128, KT, 128)).astype(ml_dtypes.bfloat16)
    wf_t = np.zeros((2, 128, KT, 386), np.float32)
    for f2, (foff, fsz) in enumerate(((0, 384), (384, 386))):
        wf_t[f2, :, :, :fsz] = wf_aug[:, foff:foff + fsz].reshape(
            KT, 128, fsz).transpose(1, 0, 2)
    wf_t = wf_t.astype(ml_dtypes.bfloat16)

    in_maps = []
    for c in range(N_CORES):
        xl = x[BL * c:BL * (c + 1)]
        xT = xl.transpose(0, 2, 1)  # [BL, D, S]
        # [BL, th, kt, 128, 512]: one contiguous [128, 512] block per DMA
        xT_t = np.ascontiguousarray(
            xT.reshape(BL, KT, 128, 2, 512).transpose(0, 3, 1, 2, 4)).astype(
                ml_dtypes.bfloat16)
        in_maps.append({
            "xTt": xT_t, "wqkt": wqk_t, "wft": wf_t,
            "onesc": np.ones((128, 1), ml_dtypes.bfloat16),
            "boute": boute_arr,
        })
    return in_maps


def _get_prog():
    global _prog
    if _prog is None:
        _prog = _build()
    return _prog


def _run(in_maps, **kwargs):
    res = run_bass_kernel_spmd(_get_prog(), in_maps, list(range(N_CORES)), **kwargs)
    return res


def kernel(x, w_qkv, b_qkv, w_out, b_out):
    in_maps = _prepare_in_maps(x, w_qkv, b_qkv, w_out, b_out)
    res = _run(in_maps)
    # kernel produces y transposed ([BL, D, S]); transpose back on host
    y = np.concatenate(
        [np.asarray(res.results[c]["y"]).astype(np.float32).transpose(0, 2, 1)
         for c in range(N_CORES)], axis=0)
    return np.ascontiguousarray(y)
